# revision 12
# baseline (speedup 1.0000x reference)
"""Trainium2 Bass kernel for nn_AxisNetwork (embedding_lookup + sine MLP).

Math per point (x, y):
    e = lerp(emb0, x) * lerp(emb1, y)          # [256]
    h = sin(30*(e @ w0.T + b0))                # [128]
    h = sin(30*(h @ w1.T + b1))                # [128]
    out = h @ w2.T + b2                        # [3]

Device strategy (pure data parallel over 8 cores, B = N/8 points each).

A per-point table gather is descriptor-bound on TRN2: SWDGE generates one
DMA descriptor per gathered row on the GPSIMD Q7 (~8 ns each), which
serializes at ~2 ms/core.  Instead the interpolation itself is done on the
tensor engine with NO gather at all:

  * The host sorts each core's points by x-cell into chunks of 8192, then
    by y-cell within each chunk, and greedily packs consecutive points
    into 512-point STAGES whose x- and y-index windows each fit in 44
    table rows (the input's x/y correlation makes this cheap: ~270 stages
    per core, ~5% padding).
  * Per stage the host slices the ORIGINAL (non-upsampled) 512x256
    embedding tables to the stage's windows -> winx/winy [44, 256] f16,
    and emits per-point window-relative coordinates: AC5 rows
    [i0x', wx, i0y', wy, 1] (f16; i0' is an exact small integer).
  * On device, one k=5 matmul builds M[r, n] = ac'_n - r for all 88
    window rows; DVE computes u = |M|-1, ACT computes tent = relu(-u)
    = relu(1 - |ac' - r|) -- the exact linear-interpolation weights
    (tent basis reproduces piecewise-linear interp exactly).
  * Interp = tent matmuls: e_axis[d, n] = win[., d].T @ tent (k=44).
    DVE multiplies the two axes' PSUM results into ee [256, 512] f16.
  * Sine MLP as before: z0 = w0 @ ee (k=256), ACT sin(30 z + 30 b);
    z1 with w1 pre-scaled by 30, DVE wraps into [-pi, pi] (ACT's Sin
    spline range, verified on HW), sin; z2 with points as the stationary
    dim; DVE adds b2 and the host undoes the sort permutation.
"""

import os

import numpy as np

N_FULL = 1 << 20
NCORES = 8
B = N_FULL // NCORES      # points per core
RES = 512
ED = 256
HID = 128
NOUT = 3
W0_FREQ = 30.0

CHUNK = 8192              # x-sort chunk (window fitting granularity)
STAGE = 512               # points per compute stage
WX = 63                   # x tent rows (chunk x-span <= 35 verified)
WY = 63                   # y tent rows (stages greedily cut to span <= 61)
WT = 128                  # [63 x-tents, x-sum-row, 63 y-tents, y-sum-row]
GRP = 16                  # stages per AC5 load
WGRP = 4                  # stages per window load

P = 128
SUB = STAGE // P

_cache = {}


def _build_nc(s_tot):
    import concourse.bacc as bacc
    import concourse.bass as bass
    import concourse.mybir as mybir
    import concourse.tile as tile

    f32 = mybir.dt.float32
    f16 = mybir.dt.float16
    Alu = mybir.AluOpType
    Act = mybir.ActivationFunctionType

    BS = s_tot * STAGE    # padded point slots per core

    nc = bacc.Bacc("TRN2", target_bir_lowering=False, debug=False,
                   num_devices=NCORES)

    ac5_d = nc.dram_tensor("ac5", [5, BS], f16, kind="ExternalInput")
    win_d = nc.dram_tensor("win", [s_tot, WT, ED], f16, kind="ExternalInput")
    aff_d = nc.dram_tensor("aff", [5, WT], f16, kind="ExternalInput")
    w0t_d = nc.dram_tensor("w0t", [2, P, HID], f16, kind="ExternalInput")
    w1t_d = nc.dram_tensor("w1t", [HID, HID], f16, kind="ExternalInput")
    w2t_d = nc.dram_tensor("w2t", [HID, NOUT], f16, kind="ExternalInput")
    b0s_d = nc.dram_tensor("b0s", [P, 1], f32, kind="ExternalInput")
    b1s_d = nc.dram_tensor("b1s", [P, 1], f32, kind="ExternalInput")
    b2t_d = nc.dram_tensor("b2t", [NOUT, STAGE], f32, kind="ExternalInput")
    out_d = nc.dram_tensor("out", [s_tot, NOUT, STAGE], f32,
                           kind="ExternalOutput")

    with tile.TileContext(nc) as tc:
        with (
            tc.tile_pool(name="const", bufs=1) as cpool,
            tc.tile_pool(name="acp", bufs=2) as acp,
            tc.tile_pool(name="winp", bufs=2) as winp,
            tc.tile_pool(name="tp", bufs=2) as tp,
            tc.tile_pool(name="act", bufs=2) as actp,
            tc.tile_pool(name="psM", bufs=2, space="PSUM") as psM,
            tc.tile_pool(name="psE", bufs=1, space="PSUM") as psE,
            tc.tile_pool(name="psA", bufs=1, space="PSUM") as psA,
            tc.tile_pool(name="psB", bufs=1, space="PSUM") as psB,
        ):
            # ---- constants / weights ----
            aff = cpool.tile([5, WT], f16)
            nc.sync.dma_start(out=aff[:], in_=aff_d[:])
            w0t = cpool.tile([P, 2, HID], f16)       # [k, c, m]
            nc.sync.dma_start(out=w0t[:], in_=w0t_d[:].rearrange("c k m -> k c m"))
            w1t = cpool.tile([HID, HID], f16)
            nc.sync.dma_start(out=w1t[:], in_=w1t_d[:])
            w2t = cpool.tile([HID, NOUT], f16)
            nc.sync.dma_start(out=w2t[:], in_=w2t_d[:])
            b0s = cpool.tile([P, 1], f32)
            nc.sync.dma_start(out=b0s[:], in_=b0s_d[:])
            b1s = cpool.tile([P, 1], f32)
            nc.sync.dma_start(out=b1s[:], in_=b1s_d[:])
            b2t = cpool.tile([NOUT, STAGE], f32)
            nc.sync.dma_start(out=b2t[:], in_=b2t_d[:])

            for s in range(s_tot):
                if s % GRP == 0:
                    ac5t = acp.tile([5, GRP * STAGE], f16, tag="ac5")
                    nc.sync.dma_start(
                        out=ac5t[:],
                        in_=ac5_d[:, s * STAGE:(s + GRP) * STAGE])
                if s % WGRP == 0:
                    w4 = winp.tile([WT, WGRP, ED], f16, tag="w4")
                    nc.sync.dma_start(
                        out=w4[:],
                        in_=win_d[s:s + WGRP].rearrange("s r d -> r s d"))
                off = (s % GRP) * STAGE
                wi = s % WGRP

                # tent args: M[r, n] = ac'_n - r for 88 window rows
                M2 = psM.tile([WT, STAGE], f32, tag="m2")
                nc.tensor.matmul(M2[:], aff[:], ac5t[:, off:off + STAGE],
                                 start=True, stop=True)
                # v = min(|M|, 1); tent = 1 - v is folded into the window
                # tables (negated rows + sum row against the const-2 column)
                u = tp.tile([WT, STAGE], f16, tag="u")
                nc.scalar.activation(out=u[:], in_=M2[:], func=Act.Abs)
                v = tp.tile([WT, STAGE], f16, tag="v")
                nc.vector.tensor_scalar(out=v[:], in0=u[:], scalar1=1.0,
                                        scalar2=0.0, op0=Alu.min,
                                        op1=Alu.add)

                # interpolation: e_axis[d, n] = win'[., d].T @ v
                e0 = psE.tile([P, 2, STAGE], f32, tag="e0")
                e1 = psE.tile([P, 2, STAGE], f32, tag="e1")
                for h in range(2):
                    nc.tensor.matmul(e0[:, h, :],
                                     w4[0:64, wi, h * P:(h + 1) * P],
                                     v[0:64, :], start=True, stop=True)
                    nc.tensor.matmul(e1[:, h, :],
                                     w4[64:128, wi, h * P:(h + 1) * P],
                                     v[64:128, :], start=True, stop=True)
                # DVE cannot read two PSUM operands: stage e0 through SBUF
                s0 = tp.tile([P, 2, STAGE], f16, tag="s0")
                ee = tp.tile([P, 2, STAGE], f16, tag="ee")
                for h in range(2):
                    nc.scalar.activation(out=s0[:, h, :], in_=e0[:, h, :],
                                         func=Act.Copy)
                    nc.vector.tensor_tensor(
                        out=ee[:, h, :], in0=s0[:, h, :], in1=e1[:, h, :],
                        op=Alu.mult)

                # layer 0: z0[h, n] = sum_d w0[h, d] ee[d, n]
                z0 = psA.tile([P, STAGE], f32, tag="z0")
                for c in range(2):
                    nc.tensor.matmul(z0[:], w0t[:, c, :], ee[:, c, :],
                                     start=(c == 0), stop=(c == 1))
                h0 = actp.tile([P, STAGE], f16, tag="h0")
                nc.scalar.activation(out=h0[:], in_=z0[:], func=Act.Sin,
                                     bias=b0s[:], scale=W0_FREQ)
                # layer 1 (w1t pre-scaled by 30; wrap into ACT Sin's range)
                z1 = psB.tile([P, STAGE], f32, tag="zb")
                nc.tensor.matmul(z1[:], w1t[:], h0[:], start=True, stop=True)
                t1 = actp.tile([P, STAGE], f32, tag="t1")
                nc.vector.add_range_wrap(out=t1[:], in_=z1[:], shift=b1s[:],
                                         bound=float(np.pi),
                                         period=float(2 * np.pi))
                h1 = actp.tile([P, STAGE], f16, tag="h1")
                nc.scalar.activation(out=h1[:], in_=t1[:], func=Act.Sin)
                # layer 2: w2t stationary, points stream -> o2T [3, n]
                o2 = psB.tile([NOUT, STAGE], f32, tag="zb")
                nc.tensor.matmul(o2[:], w2t[:], h1[:], start=True, stop=True)
                osb = actp.tile([NOUT, STAGE], f32, tag="osb")
                nc.vector.scalar_tensor_tensor(
                    out=osb[:], in0=o2[:], scalar=1.0, in1=b2t[:],
                    op0=Alu.mult, op1=Alu.add)
                nc.sync.dma_start(out=out_d[s], in_=osb[:])

    nc.compile()
    return nc


def _plan_core(pts):
    """Sort/bucket one core's points; returns the stage plan."""
    acx = (0.5 * np.clip(pts[:, 0].astype(np.float64), -1.0, 0.999) + 0.5) \
        * (RES - 1)
    acy = (0.5 * np.clip(pts[:, 1].astype(np.float64), -1.0, 0.999) + 0.5) \
        * (RES - 1)
    i0x = np.floor(acx).astype(np.int64)
    i0y = np.floor(acy).astype(np.int64)
    wx = acx - i0x
    wy = acy - i0y

    stages = []   # (point_idx_array, basex, basey)
    order1 = np.argsort(i0x, kind="stable")
    for k in range(B // CHUNK):
        seg = order1[k * CHUNK:(k + 1) * CHUNK]
        bx = int(i0x[seg].min())
        assert int(i0x[seg].max()) - bx + 1 <= WX, "x window overflow"
        seg2 = seg[np.argsort(i0y[seg], kind="stable")]
        sy = i0y[seg2]
        i = 0
        n = len(sy)
        while i < n:
            j = min(i + STAGE, n)
            while sy[j - 1] - sy[i] + 1 > WY - 2:
                j = i + np.searchsorted(sy[i:j], sy[i] + WY - 2,
                                        side="right")
            stages.append((seg2[i:j], bx, int(sy[i])))
            i = j
    return stages, i0x, i0y, wx, wy


def _host_prep(inputs):
    coords = np.ascontiguousarray(inputs["coords"], dtype=np.float32)
    emb0 = np.asarray(inputs["emb0"], dtype=np.float32)
    emb1 = np.asarray(inputs["emb1"], dtype=np.float32)
    w0 = np.asarray(inputs["w0"], dtype=np.float32)
    b0 = np.asarray(inputs["b0"], dtype=np.float32)
    w1 = np.asarray(inputs["w1"], dtype=np.float32)
    b1 = np.asarray(inputs["b1"], dtype=np.float32)
    w2 = np.asarray(inputs["w2"], dtype=np.float32)
    b2 = np.asarray(inputs["b2"], dtype=np.float32)

    plans = []
    max_stages = 0
    for c in range(NCORES):
        plan = _plan_core(coords[c * B:(c + 1) * B])
        plans.append(plan)
        max_stages = max(max_stages, len(plan[0]))
    lcm = np.lcm(GRP, WGRP)
    s_tot = int(-(-max_stages // lcm) * lcm)

    emb0h = emb0.astype(np.float16)
    emb1h = emb1.astype(np.float16)
    w0t = np.ascontiguousarray(
        w0.T.reshape(2, P, HID)).astype(np.float16)        # [c, k, m]
    w1t = np.ascontiguousarray(w1.T * W0_FREQ).astype(np.float16)
    w2t = np.ascontiguousarray(w2.T).astype(np.float16)
    b0s = (W0_FREQ * b0).reshape(P, 1).astype(np.float32)
    b1s = (W0_FREQ * b1).reshape(P, 1).astype(np.float32)
    b2t = np.repeat(b2.reshape(NOUT, 1), STAGE, 1).astype(np.float32)
    aff = np.zeros((5, WT), np.float32)
    aff[0, :WX] = 1.0
    aff[1, :WX] = 1.0
    aff[4, :WX] = -np.arange(WX)
    aff[4, 63] = 2.0
    aff[2, 64:64 + WY] = 1.0
    aff[3, 64:64 + WY] = 1.0
    aff[4, 64:64 + WY] = -np.arange(WY)
    aff[4, 127] = 2.0
    aff = aff.astype(np.float16)

    shared = dict(aff=aff, w0t=w0t, w1t=w1t, w2t=w2t,
                  b0s=b0s, b1s=b1s, b2t=b2t)
    in_maps = []
    perms = []
    BS = s_tot * STAGE
    for c in range(NCORES):
        stages, i0x, i0y, wx, wy = plans[c]
        ac5 = np.zeros((5, BS), np.float32)
        ac5[4] = 1.0
        win = np.zeros((s_tot, WT, ED), np.float16)
        pos = np.full(BS, -1, np.int64)     # position -> original point
        for s, (idx, bx, by) in enumerate(stages):
            n = len(idx)
            sl = slice(s * STAGE, s * STAGE + n)
            ac5[0, sl] = i0x[idx] - bx
            ac5[1, sl] = wx[idx]
            ac5[2, sl] = i0y[idx] - by
            ac5[3, sl] = wy[idx]
            pos[s * STAGE:s * STAGE + n] = idx
            nrx = min(WX, RES - bx)
            win[s, :nrx] = -emb0h[bx:bx + nrx]
            win[s, 63] = emb0.astype(np.float64)[bx:bx + nrx].sum(0).astype(
                np.float16)
            nry = min(WY, RES - by)
            win[s, 64:64 + nry] = -emb1h[by:by + nry]
            win[s, 127] = emb1.astype(np.float64)[by:by + nry].sum(0).astype(
                np.float16)
        in_maps.append(dict(ac5=ac5.astype(np.float16), win=win, **shared))
        perms.append(pos)
    return in_maps, perms, s_tot


last_results = None


def kernel(**inputs):
    global last_results
    from concourse.bass_utils import run_bass_kernel_spmd
    import os

    in_maps, perms, s_tot = _host_prep(inputs)
    key = ("nc", s_tot)
    if key not in _cache:
        _cache[key] = _build_nc(s_tot)
    nc = _cache[key]

    trace = bool(int(os.environ.get("KERNEL_TRACE", "0")))
    res = run_bass_kernel_spmd(nc, in_maps, core_ids=list(range(NCORES)),
                               trace=trace)
    last_results = res

    BS = s_tot * STAGE
    outs = []
    for c in range(NCORES):
        dev = res.results[c]["out"]                  # [s_tot, 3, 512]
        flat = dev.reshape(s_tot, NOUT, STAGE).transpose(0, 2, 1).reshape(
            BS, NOUT)                                # indexed by position
        pos = perms[c]
        valid = pos >= 0
        out_c = np.empty((B, NOUT), flat.dtype)
        out_c[pos[valid]] = flat[valid]
        outs.append(out_c)
    return np.ascontiguousarray(
        np.concatenate(outs, 0).astype(np.float32))
